# revision 9
# baseline (speedup 1.0000x reference)
"""Bass/Trainium2 kernel for nn_BezierReinforceWrapper (scatter_memory).

Strategy (pure data parallel over batch, 8 cores x 256 rows):
  - mu = x @ W + b and sample = sigmoid(mu) on device (PE + ACT).
  - Bezier points via a constant Bernstein matrix matmul (PE, fp32).
  - Rounded integer coords -> per-point one-hot "marker" tiles
    (DVE tensor_scalar is_equal / ACT Abs+Relu tent), 128 points per tile.
  - Per batch row: G = Ex^T @ Ey accumulated on PE (counts of rounded
    centers); 3x3 brush + border clipping folded in as counts = F @ G @ F
    with a constant banded matrix F (computed as two extra matmuls using
    the lhsT-transpose trick: P2 = G^T F, P3 = P2^T F = F G F).
  - canvas = Relu(0.3 - 0.07 * counts) on ACT, DMA out in 16-row blocks.
  - log_prob / entropy are input-independent constants (raw == mu) and are
    produced on the host exactly as the reference computes them.
"""

import os
import sys

import numpy as np

for _p in ("/opt/trn_rl_repo", "/root/.axon_site/_ro/trn_rl_repo"):
    if os.path.isdir(_p) and _p not in sys.path:
        sys.path.insert(0, _p)

import ml_dtypes  # noqa: E402

from concourse import bacc, mybir, tile  # noqa: E402
from concourse.bass_utils import run_bass_kernel_spmd  # noqa: E402

F32 = mybir.dt.float32
BF16 = mybir.dt.bfloat16
I32 = mybir.dt.int32
BF = ml_dtypes.bfloat16

# Problem constants (hardcoded from the module definition).
B = 2048
D = 256
S = 32
CANVAS = 128
NUM_T = 50
W_STAMP = -0.07
BG = 0.3
SCALE = 1e-4
N_CORES = 8
R = B // N_CORES          # rows per core = 256
NPTS = S * NUM_T          # 1600 points per row
NCH = (NPTS + 127) // 128  # 13 chunks of 128 points (last chunk padded)
NPAD = NCH * 128          # 1664
KDIM = 3 * S + 1          # 97 = control-point rows + bias/pad row
OUT_BLOCK = 16            # canvas rows per output DMA

_ACT_EY_CHUNKS = frozenset((11, 12))  # ey marker tiles built on ACT


def _host_constants():
    """Build the constant operand tensors shipped to every core."""
    # Bernstein coefficient matrix, mimicking the reference fp32 ops:
    # t = linspace fp32; c0=(1-t)**2, c1=2*(1-t)*t, c2=t**2, all fp32.
    t = np.linspace(0.0, 1.0, NUM_T).astype(np.float32)
    one = np.float32(1.0)
    c0 = (one - t) * (one - t)
    c1 = (np.float32(2.0) * (one - t)) * t
    c2 = t * t
    # x128 folding is exact (power of two).
    coef = np.stack([c0, c1, c2], axis=0) * np.float32(CANVAS)  # [3, 50]

    bigT = np.zeros((KDIM, NPAD), dtype=np.float32)
    for g in range(NPTS):
        s_idx, t_idx = divmod(g, NUM_T)
        for i in range(3):
            bigT[3 * s_idx + i, g] = coef[i, t_idx]
    # bias/pad row: 0 for real points, large negative for padding points so
    # their rounded coordinate matches no marker bin.
    bigT[KDIM - 1, NPTS:] = -100000.0

    iota = np.tile(np.arange(128, dtype=np.float32), (128, 1))
    iota_bf = iota.astype(BF)
    iota_neg_bf = (-iota).astype(BF)

    # Brush fold matrix: F[:, c] is the 3-wide clipped stamp for center c.
    F = np.zeros((128, 128), dtype=np.float32)
    for c in range(128):
        for d in (-1, 0, 1):
            i = c + d
            if 0 <= i < 128:
                F[i, c] += 1.0
    F[0, 0] = 2.0       # center 0: offsets {-1,0} clip to 0
    F[127, 127] = 2.0   # center 127: offsets {0,1} clip to 127
    F_bf = F.astype(BF)

    ident = np.eye(128, dtype=np.float32)
    return bigT, iota_bf, iota_neg_bf, F_bf, ident


def _build_program(n_rows=R, trace_sim=False):
    """Emit the per-core Bass program (same program for all cores)."""
    nc = bacc.Bacc("TRN2", target_bir_lowering=False, debug=False)

    xsh_d = nc.dram_tensor("xsh", [n_rows, D], F32, kind="ExternalInput")
    wb_d = nc.dram_tensor("wb", [D + 1, S * 6], F32, kind="ExternalInput")
    wbx_d = nc.dram_tensor("wbx", [D + 1, 3 * S], F32, kind="ExternalInput")
    wby_d = nc.dram_tensor("wby", [D + 1, 3 * S], F32, kind="ExternalInput")
    bigT_d = nc.dram_tensor("bigT", [KDIM, NPAD], F32, kind="ExternalInput")
    iota_d = nc.dram_tensor("iota", [128, 128], BF16, kind="ExternalInput")
    iotan_d = nc.dram_tensor("iotan", [128, 128], BF16, kind="ExternalInput")
    fmat_d = nc.dram_tensor("fmat", [128, 128], BF16, kind="ExternalInput")
    ident_d = nc.dram_tensor("ident", [128, 128], F32, kind="ExternalInput")

    sketch_d = nc.dram_tensor(
        "sketch", [n_rows, CANVAS, CANVAS], F32, kind="ExternalOutput"
    )
    sample_d = nc.dram_tensor("sample", [n_rows, S * 6], F32, kind="ExternalOutput")

    RC = (n_rows + 127) // 128  # row chunks (2 for full size)
    assert n_rows % 128 == 0 or RC == 1
    rows_pc = [min(128, n_rows - 128 * i) for i in range(RC)]
    out_blk = min(OUT_BLOCK, n_rows)
    assert n_rows % out_blk == 0

    with tile.TileContext(nc, trace_sim=trace_sim) as tc:
        with (
            tc.tile_pool(name="const", bufs=1) as cpool,
            tc.tile_pool(name="pts", bufs=1) as ppool,
            tc.tile_pool(name="work", bufs=3) as wpool,
            tc.tile_pool(name="mark", bufs=8) as mpool,
            tc.tile_pool(name="gsb", bufs=4) as gpool,
            tc.tile_pool(name="stage", bufs=2) as spool,
            tc.tile_pool(name="ps_setup", bufs=2, space="PSUM") as ps_setup,
            tc.tile_pool(name="ps_loop", bufs=2, space="PSUM") as ps_loop,
        ):
            # ---- constants into SBUF ----
            iota_sb = cpool.tile([128, 128], BF16)
            iotan_sb = cpool.tile([128, 128], BF16)
            fmat_sb = cpool.tile([128, 128], BF16)
            ident_sb = cpool.tile([128, 128], F32)
            bigT_sb = cpool.tile([KDIM, NPAD], F32)
            wb_sb = [cpool.tile([128, S * 6], F32, name=f"wb{k}") for k in range(2)]
            wb2_sb = cpool.tile([1, S * 6], F32)
            wbx_sb = [cpool.tile([128, 3 * S], F32, name=f"wbx{k}") for k in range(2)]
            wbx2_sb = cpool.tile([1, 3 * S], F32)
            wby_sb = [cpool.tile([128, 3 * S], F32, name=f"wby{k}") for k in range(2)]
            wby2_sb = cpool.tile([1, 3 * S], F32)
            ones_sb = cpool.tile([1, 256], F32)
            c1_sb = cpool.tile([128, 1], F32)
            cbg_sb = cpool.tile([128, 1], F32)

            nc.sync.dma_start(iota_sb[:], iota_d[:])
            nc.sync.dma_start(iotan_sb[:], iotan_d[:])
            nc.sync.dma_start(fmat_sb[:], fmat_d[:])
            nc.sync.dma_start(ident_sb[:], ident_d[:])
            nc.sync.dma_start(bigT_sb[:], bigT_d[:])
            for k in range(2):
                nc.sync.dma_start(wb_sb[k][:], wb_d[128 * k : 128 * (k + 1), :])
                nc.sync.dma_start(wbx_sb[k][:], wbx_d[128 * k : 128 * (k + 1), :])
                nc.sync.dma_start(wby_sb[k][:], wby_d[128 * k : 128 * (k + 1), :])
            nc.sync.dma_start(wb2_sb[:], wb_d[D : D + 1, :])
            nc.sync.dma_start(wbx2_sb[:], wbx_d[D : D + 1, :])
            nc.sync.dma_start(wby2_sb[:], wby_d[D : D + 1, :])
            nc.vector.memset(ones_sb[:], 1.0)
            nc.vector.memset(c1_sb[:], 1.0)
            nc.vector.memset(cbg_sb[:], BG)

            # ---- x and x^T ----
            x_sb = [
                [cpool.tile([128, 128], F32, name=f"x{rc}_{kc}") for kc in range(2)]
                for rc in range(RC)
            ]
            xT_sb = [cpool.tile([128, max(n_rows, 2)], F32, name=f"xT{k}") for k in range(2)]
            for rc in range(RC):
                for kc in range(2):
                    nc.sync.dma_start(
                        x_sb[rc][kc][: rows_pc[rc], :],
                        xsh_d[128 * rc : 128 * rc + rows_pc[rc],
                              128 * kc : 128 * (kc + 1)],
                    )
            for rc in range(RC):
                for kc in range(2):
                    tp = ps_setup.tile([128, 256], F32, space="PSUM", name="ps_a", tag="ps_a")
                    nc.tensor.transpose(
                        out=tp[:, : rows_pc[rc]],
                        in_=x_sb[rc][kc][: rows_pc[rc], :],
                        identity=ident_sb[: rows_pc[rc], : rows_pc[rc]],
                    )
                    nc.vector.tensor_copy(
                        out=xT_sb[kc][:, 128 * rc : 128 * rc + rows_pc[rc]],
                        in_=tp[:, : rows_pc[rc]],
                    )

            # ---- mu -> sigmoid -> sample (row-major) ----
            for rc in range(RC):
                nr = rows_pc[rc]
                mu_ps = ps_setup.tile([128, 256], F32, space="PSUM", name="ps_a", tag="ps_a")
                for kc in range(2):
                    nc.tensor.matmul(
                        out=mu_ps[:nr, : S * 6],
                        lhsT=xT_sb[kc][:, 128 * rc : 128 * rc + nr],
                        rhs=wb_sb[kc][:],
                        start=(kc == 0),
                        stop=False,
                    )
                nc.tensor.matmul(
                    out=mu_ps[:nr, : S * 6],
                    lhsT=ones_sb[0:1, 128 * rc : 128 * rc + nr],
                    rhs=wb2_sb[0:1, :],
                    start=False,
                    stop=True,
                )
                samp_sb = wpool.tile([128, S * 6], F32, name="samp", tag="samp")
                nc.scalar.activation(
                    out=samp_sb[:nr, :],
                    in_=mu_ps[:nr, : S * 6],
                    func=mybir.ActivationFunctionType.Sigmoid,
                )
                nc.sync.dma_start(
                    sample_d[128 * rc : 128 * rc + nr, :], samp_sb[:nr, :]
                )

            # ---- mu^T (x/y control points) -> sigmoid -> cpT ----
            cpT_sb = [cpool.tile([KDIM, max(n_rows, 2)], F32, name=f"cpT{k}") for k in range(2)]
            for coord, (wc, wc2) in enumerate(
                ((wbx_sb, wbx2_sb), (wby_sb, wby2_sb))
            ):
                nc.vector.memset(cpT_sb[coord][KDIM - 1 : KDIM, :], 1.0)
                muT_ps = ps_setup.tile([128, 256], F32, space="PSUM", name="ps_a", tag="ps_a")
                for kc in range(2):
                    nc.tensor.matmul(
                        out=muT_ps[: 3 * S, :n_rows],
                        lhsT=wc[kc][:],
                        rhs=xT_sb[kc][:, :n_rows],
                        start=(kc == 0),
                        stop=False,
                    )
                nc.tensor.matmul(
                    out=muT_ps[: 3 * S, :n_rows],
                    lhsT=wc2[0:1, :],
                    rhs=ones_sb[0:1, :n_rows],
                    start=False,
                    stop=True,
                )
                nc.scalar.activation(
                    out=cpT_sb[coord][: 3 * S, :n_rows],
                    in_=muT_ps[: 3 * S, :n_rows],
                    func=mybir.ActivationFunctionType.Sigmoid,
                )

            # ---- Bezier points -> rounded integer coords (as f32) ----
            pts = [
                [ppool.tile([128, max(n_rows, 2)], F32, name=f"pts{coord}_{c}")
                 for c in range(NCH)]
                for coord in range(2)
            ]
            for coord in range(2):
                for c in range(NCH):
                    bz_ps = ps_setup.tile([128, 256], F32, space="PSUM", name="ps_a", tag="ps_a")
                    nc.tensor.matmul(
                        out=bz_ps[:, :n_rows],
                        lhsT=bigT_sb[:, 128 * c : 128 * (c + 1)],
                        rhs=cpT_sb[coord][:, :n_rows],
                        start=True,
                        stop=True,
                    )
                    pi = wpool.tile([128, max(n_rows, 2)], I32, name="ptsi", tag="ptsi")
                    # HW f32->i32 cast rounds to nearest even == jnp.round.
                    # Clamp high side so a (never observed) 127.5+ coordinate
                    # folds onto 127 instead of escaping the marker range.
                    nc.vector.tensor_scalar(
                        out=pi[:, :n_rows],
                        in0=bz_ps[:, :n_rows],
                        scalar1=127.4375,
                        scalar2=None,
                        op0=mybir.AluOpType.min,
                    )
                    nc.vector.tensor_copy(out=pts[coord][c][:, :n_rows], in_=pi[:, :n_rows])

            # ---- main per-row loop ----
            stg = None
            for r in range(n_rows):
                g_ps = ps_loop.tile([128, 128], F32, space="PSUM", name="G", tag="G")
                for c in range(NCH):
                    ex = mpool.tile([128, 128], BF16, name="ex", tag="ex")
                    nc.vector.tensor_scalar(
                        out=ex[:],
                        in0=iota_sb[:],
                        scalar1=pts[0][c][:, r : r + 1],
                        scalar2=None,
                        op0=mybir.AluOpType.is_equal,
                    )
                    ey = mpool.tile([128, 128], BF16, name="ey", tag="ey")
                    if c in _ACT_EY_CHUNKS:
                        tmp = mpool.tile([128, 128], BF16, name="tmpm", tag="tmpm")
                        nc.scalar.activation(
                            out=tmp[:],
                            in_=iotan_sb[:],
                            func=mybir.ActivationFunctionType.Abs,
                            bias=pts[1][c][:, r : r + 1],
                        )
                        nc.scalar.activation(
                            out=ey[:],
                            in_=tmp[:],
                            func=mybir.ActivationFunctionType.Relu,
                            bias=c1_sb[:, 0:1],
                            scale=-1.0,
                        )
                    else:
                        nc.vector.tensor_scalar(
                            out=ey[:],
                            in0=iota_sb[:],
                            scalar1=pts[1][c][:, r : r + 1],
                            scalar2=None,
                            op0=mybir.AluOpType.is_equal,
                        )
                    nc.tensor.matmul(
                        out=g_ps[:],
                        lhsT=ex[:],
                        rhs=ey[:],
                        start=(c == 0),
                        stop=(c == NCH - 1),
                    )
                g_sb = gpool.tile([128, 128], BF16, name="gsb", tag="gsb")
                nc.scalar.activation(
                    out=g_sb[:], in_=g_ps[:],
                    func=mybir.ActivationFunctionType.Copy,
                )
                p2_ps = ps_loop.tile([128, 128], F32, space="PSUM", name="P2", tag="P2")
                nc.tensor.matmul(
                    out=p2_ps[:], lhsT=g_sb[:], rhs=fmat_sb[:], start=True, stop=True
                )
                p2_sb = gpool.tile([128, 128], BF16, name="p2sb", tag="p2sb")
                nc.scalar.activation(
                    out=p2_sb[:], in_=p2_ps[:],
                    func=mybir.ActivationFunctionType.Copy,
                )
                p3_ps = ps_loop.tile([128, 128], F32, space="PSUM", name="P3", tag="P3")
                nc.tensor.matmul(
                    out=p3_ps[:], lhsT=p2_sb[:], rhs=fmat_sb[:], start=True, stop=True
                )
                slot = r % out_blk
                if slot == 0:
                    stg = spool.tile([128, out_blk * 128], F32, name="stage", tag="stage")
                nc.scalar.activation(
                    out=stg[:, 128 * slot : 128 * (slot + 1)],
                    in_=p3_ps[:],
                    func=mybir.ActivationFunctionType.Relu,
                    bias=cbg_sb[:, 0:1],
                    scale=W_STAMP,
                )
                if slot == out_blk - 1:
                    r0 = r - out_blk + 1
                    nc.sync.dma_start(
                        out=sketch_d[r0 : r + 1, :, :].rearrange("r x y -> x r y"),
                        in_=stg[:].rearrange("p (r y) -> p r y", r=out_blk),
                    )
    nc.compile()
    return nc


_CACHE = {}


def _get_program(n_rows=R):
    if n_rows not in _CACHE:
        _CACHE[n_rows] = _build_program(n_rows)
    return _CACHE[n_rows]


def _log2pi_f32():
    return np.log(np.float32(2.0) * np.float32(np.pi)).astype(np.float32)


def host_logprob_entropy():
    """Input-independent log_prob / entropy, fp32 like the reference."""
    scale = np.float32(SCALE)
    log2pi = _log2pi_f32()
    dim = S * 6
    term = (
        np.float32(-0.0)
        - np.log(scale).astype(np.float32)
        - np.float32(0.5) * log2pi
    ).astype(np.float32)
    log_prob = np.sum(np.full((dim,), term, dtype=np.float32)).astype(np.float32)
    ent = (
        np.float32(dim)
        * (np.float32(0.5) + np.float32(0.5) * log2pi + np.log(scale).astype(np.float32))
    ).astype(np.float32)
    return (
        np.full((B,), log_prob, dtype=np.float32),
        np.full((B,), ent, dtype=np.float32),
    )


def make_in_maps(x, W_agent, b_agent, n_rows=R, n_cores=N_CORES):
    x = np.asarray(x, dtype=np.float32)
    W = np.asarray(W_agent, dtype=np.float32)
    b = np.asarray(b_agent, dtype=np.float32)
    wb = np.concatenate([W, b[None, :]], axis=0)
    bigT, iota_bf, iotan_bf, F_bf, ident = _host_constants()
    common = {
        "wb": np.ascontiguousarray(wb),
        "wbx": np.ascontiguousarray(wb[:, 0::2]),
        "wby": np.ascontiguousarray(wb[:, 1::2]),
        "bigT": bigT,
        "iota": iota_bf,
        "iotan": iotan_bf,
        "fmat": F_bf,
        "ident": ident,
    }
    maps = []
    for ci in range(n_cores):
        m = dict(common)
        m["xsh"] = np.ascontiguousarray(x[ci * n_rows : ci * n_rows + n_rows])
        maps.append(m)
    return maps


def kernel(x, W_agent, b_agent):
    """Full-input, full-output entry point (tuple matching reference())."""
    nc = _get_program(R)
    in_maps = make_in_maps(x, W_agent, b_agent)
    res = run_bass_kernel_spmd(nc, in_maps, list(range(N_CORES)))
    sketch = np.concatenate(
        [np.asarray(res.results[i]["sketch"]) for i in range(N_CORES)], axis=0
    )
    sample = np.concatenate(
        [np.asarray(res.results[i]["sample"]) for i in range(N_CORES)], axis=0
    )
    log_prob, entropy = host_logprob_entropy()
    return (
        sketch.astype(np.float32),
        log_prob,
        entropy,
        sample.astype(np.float32),
    )


# revision 11
# speedup vs baseline: 1.5599x; 1.5599x over previous
"""Bass/Trainium2 kernel for nn_BezierReinforceWrapper (scatter_memory).

Strategy (pure data parallel over batch, 8 cores x 256 rows):
  - mu = x @ W + b and sample = sigmoid(mu) on device (PE + ACT).
  - Bezier points via a constant Bernstein matrix matmul (PE, fp32).
  - Rounded integer coords -> per-point one-hot "marker" tiles
    (DVE tensor_scalar is_equal / ACT Abs+Relu tent), 128 points per tile.
  - Per batch row: G = Ex^T @ Ey accumulated on PE (counts of rounded
    centers); 3x3 brush + border clipping folded in as counts = F @ G @ F
    with a constant banded matrix F (computed as two extra matmuls using
    the lhsT-transpose trick: P2 = G^T F, P3 = P2^T F = F G F).
  - canvas = Relu(0.3 - 0.07 * counts) on ACT, DMA out in 16-row blocks.
  - log_prob / entropy are input-independent constants (raw == mu) and are
    produced on the host exactly as the reference computes them.
"""

import os
import sys

import numpy as np

for _p in ("/opt/trn_rl_repo", "/root/.axon_site/_ro/trn_rl_repo"):
    if os.path.isdir(_p) and _p not in sys.path:
        sys.path.insert(0, _p)

import ml_dtypes  # noqa: E402

from concourse import bacc, mybir, tile  # noqa: E402
from concourse.bass_utils import run_bass_kernel_spmd  # noqa: E402

F32 = mybir.dt.float32
BF16 = mybir.dt.bfloat16
I32 = mybir.dt.int32
BF = ml_dtypes.bfloat16

# Problem constants (hardcoded from the module definition).
B = 2048
D = 256
S = 32
CANVAS = 128
NUM_T = 50
W_STAMP = -0.07
BG = 0.3
SCALE = 1e-4
N_CORES = 8
R = B // N_CORES          # rows per core = 256
NPTS = S * NUM_T          # 1600 points per row
NCH = (NPTS + 127) // 128  # 13 chunks of 128 points (last chunk padded)
NPAD = NCH * 128          # 1664
KDIM = 3 * S + 1          # 97 = control-point rows + bias/pad row
OUT_BLOCK = 16            # canvas rows per output DMA

_ACT_EY_CHUNKS = frozenset()  # ey marker tiles built on ACT
_GP_EY_CHUNKS = frozenset(range(13))  # ey marker tiles built on GPSIMD


def _host_constants():
    """Build the constant operand tensors shipped to every core."""
    # Bernstein coefficient matrix, mimicking the reference fp32 ops:
    # t = linspace fp32; c0=(1-t)**2, c1=2*(1-t)*t, c2=t**2, all fp32.
    t = np.linspace(0.0, 1.0, NUM_T).astype(np.float32)
    one = np.float32(1.0)
    c0 = (one - t) * (one - t)
    c1 = (np.float32(2.0) * (one - t)) * t
    c2 = t * t
    # x128 folding is exact (power of two).
    coef = np.stack([c0, c1, c2], axis=0) * np.float32(CANVAS)  # [3, 50]

    bigT = np.zeros((KDIM, NPAD), dtype=np.float32)
    for g in range(NPTS):
        s_idx, t_idx = divmod(g, NUM_T)
        for i in range(3):
            bigT[3 * s_idx + i, g] = coef[i, t_idx]
    # bias/pad row: 0 for real points, large negative for padding points so
    # their rounded coordinate matches no marker bin.
    bigT[KDIM - 1, NPTS:] = -100000.0

    iota = np.tile(np.arange(128, dtype=np.float32), (128, 1))
    iota_bf = iota.astype(BF)
    iota_neg_bf = (-iota).astype(BF)

    # Brush fold matrix: F[:, c] is the 3-wide clipped stamp for center c.
    F = np.zeros((128, 128), dtype=np.float32)
    for c in range(128):
        for d in (-1, 0, 1):
            i = c + d
            if 0 <= i < 128:
                F[i, c] += 1.0
    F[0, 0] = 2.0       # center 0: offsets {-1,0} clip to 0
    F[127, 127] = 2.0   # center 127: offsets {0,1} clip to 127
    F_bf = F.astype(BF)

    ident = np.eye(128, dtype=np.float32)
    return bigT, iota_bf, iota_neg_bf, F_bf, ident


def _build_program(n_rows=R, trace_sim=False):
    """Emit the per-core Bass program (same program for all cores)."""
    nc = bacc.Bacc("TRN2", target_bir_lowering=False, debug=False)

    xsh_d = nc.dram_tensor("xsh", [n_rows, D], F32, kind="ExternalInput")
    wb_d = nc.dram_tensor("wb", [D + 1, S * 6], F32, kind="ExternalInput")
    wbx_d = nc.dram_tensor("wbx", [D + 1, 3 * S], F32, kind="ExternalInput")
    wby_d = nc.dram_tensor("wby", [D + 1, 3 * S], F32, kind="ExternalInput")
    bigT_d = nc.dram_tensor("bigT", [KDIM, NPAD], F32, kind="ExternalInput")
    iota_d = nc.dram_tensor("iota", [128, 128], BF16, kind="ExternalInput")
    iotan_d = nc.dram_tensor("iotan", [128, 128], BF16, kind="ExternalInput")
    fmat_d = nc.dram_tensor("fmat", [128, 128], BF16, kind="ExternalInput")
    ident_d = nc.dram_tensor("ident", [128, 128], F32, kind="ExternalInput")

    sketch_d = nc.dram_tensor(
        "sketch", [n_rows, CANVAS, CANVAS], F32, kind="ExternalOutput"
    )
    sample_d = nc.dram_tensor("sample", [n_rows, S * 6], F32, kind="ExternalOutput")

    RC = (n_rows + 127) // 128  # row chunks (2 for full size)
    assert n_rows % 128 == 0 or RC == 1
    rows_pc = [min(128, n_rows - 128 * i) for i in range(RC)]
    out_blk = min(OUT_BLOCK, n_rows)
    assert n_rows % out_blk == 0

    with tile.TileContext(nc, trace_sim=trace_sim) as tc:
        with (
            tc.tile_pool(name="const", bufs=1) as cpool,
            tc.tile_pool(name="pts", bufs=1) as ppool,
            tc.tile_pool(name="work", bufs=3) as wpool,
            tc.tile_pool(name="mark", bufs=8) as mpool,
            tc.tile_pool(name="gsb", bufs=4) as gpool,
            tc.tile_pool(name="stage", bufs=2) as spool,
            tc.tile_pool(name="ps_setup", bufs=2, space="PSUM") as ps_setup,
            tc.tile_pool(name="ps_loop", bufs=2, space="PSUM") as ps_loop,
        ):
            # ---- constants into SBUF ----
            iota_sb = cpool.tile([128, 128], BF16)
            iotan_sb = cpool.tile([128, 128], BF16)
            fmat_sb = cpool.tile([128, 128], BF16)
            ident_sb = cpool.tile([128, 128], F32)
            bigT_sb = cpool.tile([KDIM, NPAD], F32)
            wb_sb = [cpool.tile([128, S * 6], F32, name=f"wb{k}") for k in range(2)]
            wb2_sb = cpool.tile([1, S * 6], F32)
            wbx_sb = [cpool.tile([128, 3 * S], F32, name=f"wbx{k}") for k in range(2)]
            wbx2_sb = cpool.tile([1, 3 * S], F32)
            wby_sb = [cpool.tile([128, 3 * S], F32, name=f"wby{k}") for k in range(2)]
            wby2_sb = cpool.tile([1, 3 * S], F32)
            ones_sb = cpool.tile([1, 256], F32)
            c1_sb = cpool.tile([128, 1], F32)
            cbg_sb = cpool.tile([128, 1], F32)

            nc.sync.dma_start(iota_sb[:], iota_d[:])
            nc.sync.dma_start(iotan_sb[:], iotan_d[:])
            nc.sync.dma_start(fmat_sb[:], fmat_d[:])
            nc.sync.dma_start(ident_sb[:], ident_d[:])
            nc.sync.dma_start(bigT_sb[:], bigT_d[:])
            for k in range(2):
                nc.sync.dma_start(wb_sb[k][:], wb_d[128 * k : 128 * (k + 1), :])
                nc.sync.dma_start(wbx_sb[k][:], wbx_d[128 * k : 128 * (k + 1), :])
                nc.sync.dma_start(wby_sb[k][:], wby_d[128 * k : 128 * (k + 1), :])
            nc.sync.dma_start(wb2_sb[:], wb_d[D : D + 1, :])
            nc.sync.dma_start(wbx2_sb[:], wbx_d[D : D + 1, :])
            nc.sync.dma_start(wby2_sb[:], wby_d[D : D + 1, :])
            nc.vector.memset(ones_sb[:], 1.0)
            nc.vector.memset(c1_sb[:], 1.0)
            nc.vector.memset(cbg_sb[:], BG)

            # ---- x and x^T ----
            x_sb = [
                [cpool.tile([128, 128], F32, name=f"x{rc}_{kc}") for kc in range(2)]
                for rc in range(RC)
            ]
            xT_sb = [cpool.tile([128, max(n_rows, 2)], F32, name=f"xT{k}") for k in range(2)]
            for rc in range(RC):
                for kc in range(2):
                    nc.sync.dma_start(
                        x_sb[rc][kc][: rows_pc[rc], :],
                        xsh_d[128 * rc : 128 * rc + rows_pc[rc],
                              128 * kc : 128 * (kc + 1)],
                    )
            for rc in range(RC):
                for kc in range(2):
                    tp = ps_setup.tile([128, 256], F32, space="PSUM", name="ps_a", tag="ps_a")
                    nc.tensor.transpose(
                        out=tp[:, : rows_pc[rc]],
                        in_=x_sb[rc][kc][: rows_pc[rc], :],
                        identity=ident_sb[: rows_pc[rc], : rows_pc[rc]],
                    )
                    nc.vector.tensor_copy(
                        out=xT_sb[kc][:, 128 * rc : 128 * rc + rows_pc[rc]],
                        in_=tp[:, : rows_pc[rc]],
                    )

            # ---- mu -> sigmoid -> sample (row-major) ----
            for rc in range(RC):
                nr = rows_pc[rc]
                mu_ps = ps_setup.tile([128, 256], F32, space="PSUM", name="ps_a", tag="ps_a")
                for kc in range(2):
                    nc.tensor.matmul(
                        out=mu_ps[:nr, : S * 6],
                        lhsT=xT_sb[kc][:, 128 * rc : 128 * rc + nr],
                        rhs=wb_sb[kc][:],
                        start=(kc == 0),
                        stop=False,
                    )
                nc.tensor.matmul(
                    out=mu_ps[:nr, : S * 6],
                    lhsT=ones_sb[0:1, 128 * rc : 128 * rc + nr],
                    rhs=wb2_sb[0:1, :],
                    start=False,
                    stop=True,
                )
                samp_sb = wpool.tile([128, S * 6], F32, name="samp", tag="samp")
                nc.scalar.activation(
                    out=samp_sb[:nr, :],
                    in_=mu_ps[:nr, : S * 6],
                    func=mybir.ActivationFunctionType.Sigmoid,
                )
                nc.sync.dma_start(
                    sample_d[128 * rc : 128 * rc + nr, :], samp_sb[:nr, :]
                )

            # ---- mu^T (x/y control points) -> sigmoid -> cpT ----
            cpT_sb = [cpool.tile([KDIM, max(n_rows, 2)], F32, name=f"cpT{k}") for k in range(2)]
            for coord, (wc, wc2) in enumerate(
                ((wbx_sb, wbx2_sb), (wby_sb, wby2_sb))
            ):
                nc.vector.memset(cpT_sb[coord][KDIM - 1 : KDIM, :], 1.0)
                muT_ps = ps_setup.tile([128, 256], F32, space="PSUM", name="ps_a", tag="ps_a")
                for kc in range(2):
                    nc.tensor.matmul(
                        out=muT_ps[: 3 * S, :n_rows],
                        lhsT=wc[kc][:],
                        rhs=xT_sb[kc][:, :n_rows],
                        start=(kc == 0),
                        stop=False,
                    )
                nc.tensor.matmul(
                    out=muT_ps[: 3 * S, :n_rows],
                    lhsT=wc2[0:1, :],
                    rhs=ones_sb[0:1, :n_rows],
                    start=False,
                    stop=True,
                )
                nc.scalar.activation(
                    out=cpT_sb[coord][: 3 * S, :n_rows],
                    in_=muT_ps[: 3 * S, :n_rows],
                    func=mybir.ActivationFunctionType.Sigmoid,
                )

            # ---- Bezier points -> rounded integer coords (as f32) ----
            pts = [
                [ppool.tile([128, max(n_rows, 2)], F32, name=f"pts{coord}_{c}")
                 for c in range(NCH)]
                for coord in range(2)
            ]
            for coord in range(2):
                for c in range(NCH):
                    bz_ps = ps_setup.tile([128, 256], F32, space="PSUM", name="ps_a", tag="ps_a")
                    nc.tensor.matmul(
                        out=bz_ps[:, :n_rows],
                        lhsT=bigT_sb[:, 128 * c : 128 * (c + 1)],
                        rhs=cpT_sb[coord][:, :n_rows],
                        start=True,
                        stop=True,
                    )
                    pi = wpool.tile([128, max(n_rows, 2)], I32, name="ptsi", tag="ptsi")
                    # HW f32->i32 cast rounds to nearest even == jnp.round.
                    # Clamp high side so a (never observed) 127.5+ coordinate
                    # folds onto 127 instead of escaping the marker range.
                    nc.vector.tensor_scalar(
                        out=pi[:, :n_rows],
                        in0=bz_ps[:, :n_rows],
                        scalar1=127.4375,
                        scalar2=None,
                        op0=mybir.AluOpType.min,
                    )
                    nc.vector.tensor_copy(out=pts[coord][c][:, :n_rows], in_=pi[:, :n_rows])

            # ---- main per-row loop ----
            stg = None
            for r in range(n_rows):
                g_ps = ps_loop.tile([128, 128], F32, space="PSUM", name="G", tag="G")
                for c in range(NCH):
                    ex = mpool.tile([128, 128], BF16, name="ex", tag="ex")
                    nc.vector.tensor_scalar(
                        out=ex[:],
                        in0=iota_sb[:],
                        scalar1=pts[0][c][:, r : r + 1],
                        scalar2=None,
                        op0=mybir.AluOpType.is_equal,
                    )
                    ey = mpool.tile([128, 128], BF16, name="ey", tag="ey")
                    if c in _GP_EY_CHUNKS:
                        nc.gpsimd.tensor_scalar(
                            out=ey[:],
                            in0=iota_sb[:],
                            scalar1=pts[1][c][:, r : r + 1],
                            scalar2=None,
                            op0=mybir.AluOpType.is_equal,
                        )
                    elif c in _ACT_EY_CHUNKS:
                        tmp = mpool.tile([128, 128], BF16, name="tmpm", tag="tmpm")
                        nc.scalar.activation(
                            out=tmp[:],
                            in_=iotan_sb[:],
                            func=mybir.ActivationFunctionType.Abs,
                            bias=pts[1][c][:, r : r + 1],
                        )
                        nc.scalar.activation(
                            out=ey[:],
                            in_=tmp[:],
                            func=mybir.ActivationFunctionType.Relu,
                            bias=c1_sb[:, 0:1],
                            scale=-1.0,
                        )
                    else:
                        nc.vector.tensor_scalar(
                            out=ey[:],
                            in0=iota_sb[:],
                            scalar1=pts[1][c][:, r : r + 1],
                            scalar2=None,
                            op0=mybir.AluOpType.is_equal,
                        )
                    nc.tensor.matmul(
                        out=g_ps[:],
                        lhsT=ex[:],
                        rhs=ey[:],
                        start=(c == 0),
                        stop=(c == NCH - 1),
                    )
                g_sb = gpool.tile([128, 128], BF16, name="gsb", tag="gsb")
                nc.scalar.activation(
                    out=g_sb[:], in_=g_ps[:],
                    func=mybir.ActivationFunctionType.Copy,
                )
                p2_ps = ps_loop.tile([128, 128], F32, space="PSUM", name="P2", tag="P2")
                nc.tensor.matmul(
                    out=p2_ps[:], lhsT=g_sb[:], rhs=fmat_sb[:], start=True, stop=True
                )
                p2_sb = gpool.tile([128, 128], BF16, name="p2sb", tag="p2sb")
                nc.scalar.activation(
                    out=p2_sb[:], in_=p2_ps[:],
                    func=mybir.ActivationFunctionType.Copy,
                )
                p3_ps = ps_loop.tile([128, 128], F32, space="PSUM", name="P3", tag="P3")
                nc.tensor.matmul(
                    out=p3_ps[:], lhsT=p2_sb[:], rhs=fmat_sb[:], start=True, stop=True
                )
                slot = r % out_blk
                if slot == 0:
                    stg = spool.tile([128, out_blk * 128], F32, name="stage", tag="stage")
                nc.scalar.activation(
                    out=stg[:, 128 * slot : 128 * (slot + 1)],
                    in_=p3_ps[:],
                    func=mybir.ActivationFunctionType.Relu,
                    bias=cbg_sb[:, 0:1],
                    scale=W_STAMP,
                )
                if slot == out_blk - 1:
                    r0 = r - out_blk + 1
                    nc.sync.dma_start(
                        out=sketch_d[r0 : r + 1, :, :].rearrange("r x y -> x r y"),
                        in_=stg[:].rearrange("p (r y) -> p r y", r=out_blk),
                    )
    nc.compile()
    return nc


_CACHE = {}


def _get_program(n_rows=R):
    if n_rows not in _CACHE:
        _CACHE[n_rows] = _build_program(n_rows)
    return _CACHE[n_rows]


def _log2pi_f32():
    return np.log(np.float32(2.0) * np.float32(np.pi)).astype(np.float32)


def host_logprob_entropy():
    """Input-independent log_prob / entropy, fp32 like the reference."""
    scale = np.float32(SCALE)
    log2pi = _log2pi_f32()
    dim = S * 6
    term = (
        np.float32(-0.0)
        - np.log(scale).astype(np.float32)
        - np.float32(0.5) * log2pi
    ).astype(np.float32)
    log_prob = np.sum(np.full((dim,), term, dtype=np.float32)).astype(np.float32)
    ent = (
        np.float32(dim)
        * (np.float32(0.5) + np.float32(0.5) * log2pi + np.log(scale).astype(np.float32))
    ).astype(np.float32)
    return (
        np.full((B,), log_prob, dtype=np.float32),
        np.full((B,), ent, dtype=np.float32),
    )


def make_in_maps(x, W_agent, b_agent, n_rows=R, n_cores=N_CORES):
    x = np.asarray(x, dtype=np.float32)
    W = np.asarray(W_agent, dtype=np.float32)
    b = np.asarray(b_agent, dtype=np.float32)
    wb = np.concatenate([W, b[None, :]], axis=0)
    bigT, iota_bf, iotan_bf, F_bf, ident = _host_constants()
    common = {
        "wb": np.ascontiguousarray(wb),
        "wbx": np.ascontiguousarray(wb[:, 0::2]),
        "wby": np.ascontiguousarray(wb[:, 1::2]),
        "bigT": bigT,
        "iota": iota_bf,
        "iotan": iotan_bf,
        "fmat": F_bf,
        "ident": ident,
    }
    maps = []
    for ci in range(n_cores):
        m = dict(common)
        m["xsh"] = np.ascontiguousarray(x[ci * n_rows : ci * n_rows + n_rows])
        maps.append(m)
    return maps


def kernel(x, W_agent, b_agent):
    """Full-input, full-output entry point (tuple matching reference())."""
    nc = _get_program(R)
    in_maps = make_in_maps(x, W_agent, b_agent)
    res = run_bass_kernel_spmd(nc, in_maps, list(range(N_CORES)))
    sketch = np.concatenate(
        [np.asarray(res.results[i]["sketch"]) for i in range(N_CORES)], axis=0
    )
    sample = np.concatenate(
        [np.asarray(res.results[i]["sample"]) for i in range(N_CORES)], axis=0
    )
    log_prob, entropy = host_logprob_entropy()
    return (
        sketch.astype(np.float32),
        log_prob,
        entropy,
        sample.astype(np.float32),
    )


# revision 12
# speedup vs baseline: 1.5850x; 1.0161x over previous
"""Bass/Trainium2 kernel for nn_BezierReinforceWrapper (scatter_memory).

Strategy (pure data parallel over batch, 8 cores x 256 rows):
  - mu = x @ W + b and sample = sigmoid(mu) on device (PE + ACT).
  - Bezier points via a constant Bernstein matrix matmul (PE, fp32).
  - Rounded integer coords -> per-point one-hot "marker" tiles
    (DVE tensor_scalar is_equal / ACT Abs+Relu tent), 128 points per tile.
  - Per batch row: G = Ex^T @ Ey accumulated on PE (counts of rounded
    centers); 3x3 brush + border clipping folded in as counts = F @ G @ F
    with a constant banded matrix F (computed as two extra matmuls using
    the lhsT-transpose trick: P2 = G^T F, P3 = P2^T F = F G F).
  - canvas = Relu(0.3 - 0.07 * counts) on ACT, DMA out in 16-row blocks.
  - log_prob / entropy are input-independent constants (raw == mu) and are
    produced on the host exactly as the reference computes them.
"""

import os
import sys

import numpy as np

for _p in ("/opt/trn_rl_repo", "/root/.axon_site/_ro/trn_rl_repo"):
    if os.path.isdir(_p) and _p not in sys.path:
        sys.path.insert(0, _p)

import ml_dtypes  # noqa: E402

from concourse import bacc, mybir, tile  # noqa: E402
from concourse.bass_utils import run_bass_kernel_spmd  # noqa: E402

F32 = mybir.dt.float32
BF16 = mybir.dt.bfloat16
I32 = mybir.dt.int32
BF = ml_dtypes.bfloat16

# Problem constants (hardcoded from the module definition).
B = 2048
D = 256
S = 32
CANVAS = 128
NUM_T = 50
W_STAMP = -0.07
BG = 0.3
SCALE = 1e-4
N_CORES = 8
R = B // N_CORES          # rows per core = 256
NPTS = S * NUM_T          # 1600 points per row
NCH = (NPTS + 127) // 128  # 13 chunks of 128 points (last chunk padded)
NPAD = NCH * 128          # 1664
KDIM = 3 * S + 1          # 97 = control-point rows + bias/pad row
OUT_BLOCK = 16            # canvas rows per output DMA

_ACT_EY_CHUNKS = frozenset()  # ey marker tiles built on ACT
_GP_EY_CHUNKS = frozenset(range(12))  # ey marker tiles built on GPSIMD (12); chunk 12 on DVE


def _host_constants():
    """Build the constant operand tensors shipped to every core."""
    # Bernstein coefficient matrix, mimicking the reference fp32 ops:
    # t = linspace fp32; c0=(1-t)**2, c1=2*(1-t)*t, c2=t**2, all fp32.
    t = np.linspace(0.0, 1.0, NUM_T).astype(np.float32)
    one = np.float32(1.0)
    c0 = (one - t) * (one - t)
    c1 = (np.float32(2.0) * (one - t)) * t
    c2 = t * t
    # x128 folding is exact (power of two).
    coef = np.stack([c0, c1, c2], axis=0) * np.float32(CANVAS)  # [3, 50]

    bigT = np.zeros((KDIM, NPAD), dtype=np.float32)
    for g in range(NPTS):
        s_idx, t_idx = divmod(g, NUM_T)
        for i in range(3):
            bigT[3 * s_idx + i, g] = coef[i, t_idx]
    # bias/pad row: 0 for real points, large negative for padding points so
    # their rounded coordinate matches no marker bin.
    bigT[KDIM - 1, NPTS:] = -100000.0

    iota = np.tile(np.arange(128, dtype=np.float32), (128, 1))
    iota_bf = iota.astype(BF)
    iota_neg_bf = (-iota).astype(BF)

    # Brush fold matrix: F[:, c] is the 3-wide clipped stamp for center c.
    F = np.zeros((128, 128), dtype=np.float32)
    for c in range(128):
        for d in (-1, 0, 1):
            i = c + d
            if 0 <= i < 128:
                F[i, c] += 1.0
    F[0, 0] = 2.0       # center 0: offsets {-1,0} clip to 0
    F[127, 127] = 2.0   # center 127: offsets {0,1} clip to 127
    F_bf = F.astype(BF)

    ident = np.eye(128, dtype=np.float32)
    return bigT, iota_bf, iota_neg_bf, F_bf, ident


def _build_program(n_rows=R, trace_sim=False):
    """Emit the per-core Bass program (same program for all cores)."""
    nc = bacc.Bacc("TRN2", target_bir_lowering=False, debug=False)

    xsh_d = nc.dram_tensor("xsh", [n_rows, D], F32, kind="ExternalInput")
    wb_d = nc.dram_tensor("wb", [D + 1, S * 6], F32, kind="ExternalInput")
    wbx_d = nc.dram_tensor("wbx", [D + 1, 3 * S], F32, kind="ExternalInput")
    wby_d = nc.dram_tensor("wby", [D + 1, 3 * S], F32, kind="ExternalInput")
    bigT_d = nc.dram_tensor("bigT", [KDIM, NPAD], F32, kind="ExternalInput")
    iota_d = nc.dram_tensor("iota", [128, 128], BF16, kind="ExternalInput")
    iotan_d = nc.dram_tensor("iotan", [128, 128], BF16, kind="ExternalInput")
    fmat_d = nc.dram_tensor("fmat", [128, 128], BF16, kind="ExternalInput")
    ident_d = nc.dram_tensor("ident", [128, 128], F32, kind="ExternalInput")

    sketch_d = nc.dram_tensor(
        "sketch", [n_rows, CANVAS, CANVAS], F32, kind="ExternalOutput"
    )
    sample_d = nc.dram_tensor("sample", [n_rows, S * 6], F32, kind="ExternalOutput")

    RC = (n_rows + 127) // 128  # row chunks (2 for full size)
    assert n_rows % 128 == 0 or RC == 1
    rows_pc = [min(128, n_rows - 128 * i) for i in range(RC)]
    out_blk = min(OUT_BLOCK, n_rows)
    assert n_rows % out_blk == 0

    with tile.TileContext(nc, trace_sim=trace_sim) as tc:
        with (
            tc.tile_pool(name="const", bufs=1) as cpool,
            tc.tile_pool(name="pts", bufs=1) as ppool,
            tc.tile_pool(name="work", bufs=3) as wpool,
            tc.tile_pool(name="mark", bufs=10) as mpool,
            tc.tile_pool(name="gsb", bufs=4) as gpool,
            tc.tile_pool(name="stage", bufs=2) as spool,
            tc.tile_pool(name="ps_setup", bufs=1, space="PSUM") as ps_setup,
            tc.tile_pool(name="ps_loop", bufs=2, space="PSUM") as ps_loop,
        ):
            # ---- constants into SBUF ----
            iota_sb = cpool.tile([128, 128], BF16)
            iotan_sb = cpool.tile([128, 128], BF16)
            fmat_sb = cpool.tile([128, 128], BF16)
            ident_sb = cpool.tile([128, 128], F32)
            bigT_sb = cpool.tile([KDIM, NPAD], F32)
            wb_sb = [cpool.tile([128, S * 6], F32, name=f"wb{k}") for k in range(2)]
            wb2_sb = cpool.tile([1, S * 6], F32)
            wbx_sb = [cpool.tile([128, 3 * S], F32, name=f"wbx{k}") for k in range(2)]
            wbx2_sb = cpool.tile([1, 3 * S], F32)
            wby_sb = [cpool.tile([128, 3 * S], F32, name=f"wby{k}") for k in range(2)]
            wby2_sb = cpool.tile([1, 3 * S], F32)
            ones_sb = cpool.tile([1, 256], F32)
            c1_sb = cpool.tile([128, 1], F32)
            cbg_sb = cpool.tile([128, 1], F32)

            nc.sync.dma_start(iota_sb[:], iota_d[:])
            nc.sync.dma_start(iotan_sb[:], iotan_d[:])
            nc.sync.dma_start(fmat_sb[:], fmat_d[:])
            nc.sync.dma_start(ident_sb[:], ident_d[:])
            nc.sync.dma_start(bigT_sb[:], bigT_d[:])
            for k in range(2):
                nc.sync.dma_start(wb_sb[k][:], wb_d[128 * k : 128 * (k + 1), :])
                nc.sync.dma_start(wbx_sb[k][:], wbx_d[128 * k : 128 * (k + 1), :])
                nc.sync.dma_start(wby_sb[k][:], wby_d[128 * k : 128 * (k + 1), :])
            nc.sync.dma_start(wb2_sb[:], wb_d[D : D + 1, :])
            nc.sync.dma_start(wbx2_sb[:], wbx_d[D : D + 1, :])
            nc.sync.dma_start(wby2_sb[:], wby_d[D : D + 1, :])
            nc.vector.memset(ones_sb[:], 1.0)
            nc.vector.memset(c1_sb[:], 1.0)
            nc.vector.memset(cbg_sb[:], BG)

            # ---- x and x^T ----
            x_sb = [
                [cpool.tile([128, 128], F32, name=f"x{rc}_{kc}") for kc in range(2)]
                for rc in range(RC)
            ]
            xT_sb = [cpool.tile([128, max(n_rows, 2)], F32, name=f"xT{k}") for k in range(2)]
            for rc in range(RC):
                for kc in range(2):
                    nc.sync.dma_start(
                        x_sb[rc][kc][: rows_pc[rc], :],
                        xsh_d[128 * rc : 128 * rc + rows_pc[rc],
                              128 * kc : 128 * (kc + 1)],
                    )
            for rc in range(RC):
                for kc in range(2):
                    tp = ps_setup.tile([128, 256], F32, space="PSUM", name="ps_a", tag="ps_a")
                    nc.tensor.transpose(
                        out=tp[:, : rows_pc[rc]],
                        in_=x_sb[rc][kc][: rows_pc[rc], :],
                        identity=ident_sb[: rows_pc[rc], : rows_pc[rc]],
                    )
                    nc.vector.tensor_copy(
                        out=xT_sb[kc][:, 128 * rc : 128 * rc + rows_pc[rc]],
                        in_=tp[:, : rows_pc[rc]],
                    )

            # ---- mu -> sigmoid -> sample (row-major) ----
            for rc in range(RC):
                nr = rows_pc[rc]
                mu_ps = ps_setup.tile([128, 256], F32, space="PSUM", name="ps_a", tag="ps_a")
                for kc in range(2):
                    nc.tensor.matmul(
                        out=mu_ps[:nr, : S * 6],
                        lhsT=xT_sb[kc][:, 128 * rc : 128 * rc + nr],
                        rhs=wb_sb[kc][:],
                        start=(kc == 0),
                        stop=False,
                    )
                nc.tensor.matmul(
                    out=mu_ps[:nr, : S * 6],
                    lhsT=ones_sb[0:1, 128 * rc : 128 * rc + nr],
                    rhs=wb2_sb[0:1, :],
                    start=False,
                    stop=True,
                )
                samp_sb = wpool.tile([128, S * 6], F32, name="samp", tag="samp")
                nc.scalar.activation(
                    out=samp_sb[:nr, :],
                    in_=mu_ps[:nr, : S * 6],
                    func=mybir.ActivationFunctionType.Sigmoid,
                )
                nc.sync.dma_start(
                    sample_d[128 * rc : 128 * rc + nr, :], samp_sb[:nr, :]
                )

            # ---- mu^T (x/y control points) -> sigmoid -> cpT ----
            cpT_sb = [cpool.tile([KDIM, max(n_rows, 2)], F32, name=f"cpT{k}") for k in range(2)]
            for coord, (wc, wc2) in enumerate(
                ((wbx_sb, wbx2_sb), (wby_sb, wby2_sb))
            ):
                nc.vector.memset(cpT_sb[coord][KDIM - 1 : KDIM, :], 1.0)
                muT_ps = ps_setup.tile([128, 256], F32, space="PSUM", name="ps_a", tag="ps_a")
                for kc in range(2):
                    nc.tensor.matmul(
                        out=muT_ps[: 3 * S, :n_rows],
                        lhsT=wc[kc][:],
                        rhs=xT_sb[kc][:, :n_rows],
                        start=(kc == 0),
                        stop=False,
                    )
                nc.tensor.matmul(
                    out=muT_ps[: 3 * S, :n_rows],
                    lhsT=wc2[0:1, :],
                    rhs=ones_sb[0:1, :n_rows],
                    start=False,
                    stop=True,
                )
                nc.scalar.activation(
                    out=cpT_sb[coord][: 3 * S, :n_rows],
                    in_=muT_ps[: 3 * S, :n_rows],
                    func=mybir.ActivationFunctionType.Sigmoid,
                )

            # ---- Bezier points -> rounded integer coords (as f32) ----
            pts = [
                [ppool.tile([128, max(n_rows, 2)], F32, name=f"pts{coord}_{c}")
                 for c in range(NCH)]
                for coord in range(2)
            ]
            for coord in range(2):
                for c in range(NCH):
                    bz_ps = ps_setup.tile([128, 256], F32, space="PSUM", name="ps_a", tag="ps_a")
                    nc.tensor.matmul(
                        out=bz_ps[:, :n_rows],
                        lhsT=bigT_sb[:, 128 * c : 128 * (c + 1)],
                        rhs=cpT_sb[coord][:, :n_rows],
                        start=True,
                        stop=True,
                    )
                    pi = wpool.tile([128, max(n_rows, 2)], I32, name="ptsi", tag="ptsi")
                    # HW f32->i32 cast rounds to nearest even == jnp.round.
                    # Clamp high side so a (never observed) 127.5+ coordinate
                    # folds onto 127 instead of escaping the marker range.
                    nc.vector.tensor_scalar(
                        out=pi[:, :n_rows],
                        in0=bz_ps[:, :n_rows],
                        scalar1=127.4375,
                        scalar2=None,
                        op0=mybir.AluOpType.min,
                    )
                    nc.vector.tensor_copy(out=pts[coord][c][:, :n_rows], in_=pi[:, :n_rows])

            # ---- main per-row loop ----
            stg = None
            for r in range(n_rows):
                g_ps = ps_loop.tile([128, 128], F32, space="PSUM", name="G", tag="G", bufs=3)
                for c in range(NCH):
                    ex = mpool.tile([128, 128], BF16, name="ex", tag="ex")
                    nc.vector.tensor_scalar(
                        out=ex[:],
                        in0=iota_sb[:],
                        scalar1=pts[0][c][:, r : r + 1],
                        scalar2=None,
                        op0=mybir.AluOpType.is_equal,
                    )
                    ey = mpool.tile([128, 128], BF16, name="ey", tag="ey")
                    if c in _GP_EY_CHUNKS:
                        nc.gpsimd.tensor_scalar(
                            out=ey[:],
                            in0=iota_sb[:],
                            scalar1=pts[1][c][:, r : r + 1],
                            scalar2=None,
                            op0=mybir.AluOpType.is_equal,
                        )
                    elif c in _ACT_EY_CHUNKS:
                        tmp = mpool.tile([128, 128], BF16, name="tmpm", tag="tmpm")
                        nc.scalar.activation(
                            out=tmp[:],
                            in_=iotan_sb[:],
                            func=mybir.ActivationFunctionType.Abs,
                            bias=pts[1][c][:, r : r + 1],
                        )
                        nc.scalar.activation(
                            out=ey[:],
                            in_=tmp[:],
                            func=mybir.ActivationFunctionType.Relu,
                            bias=c1_sb[:, 0:1],
                            scale=-1.0,
                        )
                    else:
                        nc.vector.tensor_scalar(
                            out=ey[:],
                            in0=iota_sb[:],
                            scalar1=pts[1][c][:, r : r + 1],
                            scalar2=None,
                            op0=mybir.AluOpType.is_equal,
                        )
                    nc.tensor.matmul(
                        out=g_ps[:],
                        lhsT=ex[:],
                        rhs=ey[:],
                        start=(c == 0),
                        stop=(c == NCH - 1),
                    )
                g_sb = gpool.tile([128, 128], BF16, name="gsb", tag="gsb")
                nc.scalar.activation(
                    out=g_sb[:], in_=g_ps[:],
                    func=mybir.ActivationFunctionType.Copy,
                )
                p2_ps = ps_loop.tile([128, 128], F32, space="PSUM", name="P2", tag="P2")
                nc.tensor.matmul(
                    out=p2_ps[:], lhsT=g_sb[:], rhs=fmat_sb[:], start=True, stop=True
                )
                p2_sb = gpool.tile([128, 128], BF16, name="p2sb", tag="p2sb")
                nc.scalar.activation(
                    out=p2_sb[:], in_=p2_ps[:],
                    func=mybir.ActivationFunctionType.Copy,
                )
                p3_ps = ps_loop.tile([128, 128], F32, space="PSUM", name="P3", tag="P3")
                nc.tensor.matmul(
                    out=p3_ps[:], lhsT=p2_sb[:], rhs=fmat_sb[:], start=True, stop=True
                )
                slot = r % out_blk
                if slot == 0:
                    stg = spool.tile([128, out_blk * 128], F32, name="stage", tag="stage")
                nc.scalar.activation(
                    out=stg[:, 128 * slot : 128 * (slot + 1)],
                    in_=p3_ps[:],
                    func=mybir.ActivationFunctionType.Relu,
                    bias=cbg_sb[:, 0:1],
                    scale=W_STAMP,
                )
                if slot == out_blk - 1:
                    r0 = r - out_blk + 1
                    nc.sync.dma_start(
                        out=sketch_d[r0 : r + 1, :, :].rearrange("r x y -> x r y"),
                        in_=stg[:].rearrange("p (r y) -> p r y", r=out_blk),
                    )
    nc.compile()
    return nc


_CACHE = {}


def _get_program(n_rows=R):
    if n_rows not in _CACHE:
        _CACHE[n_rows] = _build_program(n_rows)
    return _CACHE[n_rows]


def _log2pi_f32():
    return np.log(np.float32(2.0) * np.float32(np.pi)).astype(np.float32)


def host_logprob_entropy():
    """Input-independent log_prob / entropy, fp32 like the reference."""
    scale = np.float32(SCALE)
    log2pi = _log2pi_f32()
    dim = S * 6
    term = (
        np.float32(-0.0)
        - np.log(scale).astype(np.float32)
        - np.float32(0.5) * log2pi
    ).astype(np.float32)
    log_prob = np.sum(np.full((dim,), term, dtype=np.float32)).astype(np.float32)
    ent = (
        np.float32(dim)
        * (np.float32(0.5) + np.float32(0.5) * log2pi + np.log(scale).astype(np.float32))
    ).astype(np.float32)
    return (
        np.full((B,), log_prob, dtype=np.float32),
        np.full((B,), ent, dtype=np.float32),
    )


def make_in_maps(x, W_agent, b_agent, n_rows=R, n_cores=N_CORES):
    x = np.asarray(x, dtype=np.float32)
    W = np.asarray(W_agent, dtype=np.float32)
    b = np.asarray(b_agent, dtype=np.float32)
    wb = np.concatenate([W, b[None, :]], axis=0)
    bigT, iota_bf, iotan_bf, F_bf, ident = _host_constants()
    common = {
        "wb": np.ascontiguousarray(wb),
        "wbx": np.ascontiguousarray(wb[:, 0::2]),
        "wby": np.ascontiguousarray(wb[:, 1::2]),
        "bigT": bigT,
        "iota": iota_bf,
        "iotan": iotan_bf,
        "fmat": F_bf,
        "ident": ident,
    }
    maps = []
    for ci in range(n_cores):
        m = dict(common)
        m["xsh"] = np.ascontiguousarray(x[ci * n_rows : ci * n_rows + n_rows])
        maps.append(m)
    return maps


def kernel(x, W_agent, b_agent):
    """Full-input, full-output entry point (tuple matching reference())."""
    nc = _get_program(R)
    in_maps = make_in_maps(x, W_agent, b_agent)
    res = run_bass_kernel_spmd(nc, in_maps, list(range(N_CORES)))
    sketch = np.concatenate(
        [np.asarray(res.results[i]["sketch"]) for i in range(N_CORES)], axis=0
    )
    sample = np.concatenate(
        [np.asarray(res.results[i]["sample"]) for i in range(N_CORES)], axis=0
    )
    log_prob, entropy = host_logprob_entropy()
    return (
        sketch.astype(np.float32),
        log_prob,
        entropy,
        sample.astype(np.float32),
    )
